# revision 5
# baseline (speedup 1.0000x reference)
"""GCN layer (GPSLayer) on 8 TRN2 NeuronCores via Bass/Tile.

Math (matches reference):
  deg[d]  = #incoming edges (incl. self loop)
  dinv    = deg^-1/2
  out[d]  = dinv[d] * sum_{e: dst=d} (dinv[src] * x[src]) @ W_gcn
            + pos[d] @ W_pos + b_gcn + b_pos
Linearity lets us aggregate raw (pre-scaled) x rows first and apply the
64x64 weight matmul only to the 12.5k aggregated rows per core.

Sharding: nodes (and their incoming edges) are range-partitioned across
8 cores; x (scaled by dinv[src], fp16) is replicated so the per-edge
source gather is core-local via indirect DMA.
"""

import numpy as np

from concourse import bacc, bass, mybir
import concourse.tile as tile
from concourse.bass import IndirectOffsetOnAxis
from concourse.bass_utils import run_bass_kernel_spmd
from concourse.masks import make_identity

N_NODES = 100000
D = 64
N_CORES = 8
NPC = N_NODES // N_CORES          # 12500 nodes per core
P = 128
N_TILES = (NPC + P - 1) // P      # 98 (last tile 84 rows)
NODES_PAD = N_TILES * P           # 12544
XS_ROWS = 100096                  # 782*128; rows >= N_NODES are zero (pad target)
PAD_SRC = N_NODES                 # gather index for padding edges -> zero row
SLAB = 512                        # chunk columns per index-slab load

F16 = mybir.dt.float16
F32 = mybir.dt.float32
I32 = mybir.dt.int32


def _preprocess(x, edge_index, pos_encoding, W_gcn, b_gcn, W_pos, b_pos):
    src = np.asarray(edge_index[0], dtype=np.int64)
    dst = np.asarray(edge_index[1], dtype=np.int64)

    deg = np.bincount(dst, minlength=N_NODES).astype(np.float64) + 1.0
    dinv = (1.0 / np.sqrt(deg)).astype(np.float32)

    loop = np.arange(N_NODES, dtype=np.int64)
    src = np.concatenate([src, loop])
    dst = np.concatenate([dst, loop])

    order = np.argsort(dst, kind="stable")
    src = src[order].astype(np.int32)
    dst = dst[order]

    x_s = np.zeros((XS_ROWS, D), np.float16)
    x_s[:N_NODES] = (np.asarray(x, np.float32) * dinv[:, None]).astype(np.float16)

    # tile boundaries: core c, tile t covers dst [c*NPC + t*P, c*NPC + min((t+1)*P, NPC))
    bounds = np.empty(N_CORES * N_TILES + 1, np.int64)
    k = 0
    for c in range(N_CORES):
        for t in range(N_TILES):
            bounds[k] = c * NPC + min(t * P, NPC)
            k += 1
    bounds[-1] = N_NODES
    starts = np.searchsorted(dst, bounds)          # [784+1]
    counts = (starts[1:] - starts[:-1]).reshape(N_CORES, N_TILES)

    ct = np.maximum(1, (counts.max(axis=0) + P - 1) // P)  # chunks per tile (shared)
    off = np.zeros(N_TILES + 1, np.int64)
    np.cumsum(ct, out=off[1:])
    c_tot = int(off[-1])

    # per-edge placement (vectorized)
    g_id = np.searchsorted(bounds, dst, side="right") - 1   # group per edge
    pos_in = np.arange(len(dst)) - starts[g_id]
    t_of = g_id % N_TILES
    col = off[t_of] + pos_in // P
    row = pos_in % P
    rel = (dst - bounds[g_id]).astype(np.float16)

    src_chunks = []
    rel_chunks = []
    dinv_tiles = []
    posT_list = []
    pos_f = np.asarray(pos_encoding, np.float32)
    for c in range(N_CORES):
        lo, hi = starts[c * N_TILES], starts[(c + 1) * N_TILES]
        sc = np.full((P, c_tot), PAD_SRC, np.int32)
        rc = np.zeros((P, c_tot), np.float16)
        sc[row[lo:hi], col[lo:hi]] = src[lo:hi]
        rc[row[lo:hi], col[lo:hi]] = rel[lo:hi]
        src_chunks.append(np.ascontiguousarray(sc))
        rel_chunks.append(np.ascontiguousarray(rc))

        dv = np.zeros(NODES_PAD, np.float32)
        dv[:NPC] = dinv[c * NPC:(c + 1) * NPC]
        dinv_tiles.append(np.ascontiguousarray(dv.reshape(N_TILES, P).T))

        pa = np.zeros((65, NODES_PAD), np.float16)
        pa[:D, :NPC] = pos_f[c * NPC:(c + 1) * NPC].T.astype(np.float16)
        pa[D, :NPC] = 1.0
        posT_list.append(np.ascontiguousarray(pa))

    b_sum = (np.asarray(b_gcn, np.float32) + np.asarray(b_pos, np.float32))
    W_aug = np.zeros((65, D), np.float16)
    W_aug[:D] = np.asarray(W_pos, np.float32).astype(np.float16)
    W_aug[D] = b_sum.astype(np.float16)
    Wg16 = np.asarray(W_gcn, np.float32).astype(np.float16)

    shared = dict(x_s=x_s, W_gcn=Wg16, W_aug=W_aug)
    per_core = [
        dict(src_chunks=src_chunks[c], rel_chunks=rel_chunks[c],
             dinv_tiles=dinv_tiles[c], posT=posT_list[c])
        for c in range(N_CORES)
    ]
    return shared, per_core, ct, off, c_tot


def _build_program(ct, off, c_tot):
    nc = bacc.Bacc("TRN2", target_bir_lowering=False, debug=False)
    xs_d = nc.declare_dram_parameter("x_s", [XS_ROWS, D], F16, isOutput=False)
    src_d = nc.declare_dram_parameter("src_chunks", [P, c_tot], I32, isOutput=False)
    rel_d = nc.declare_dram_parameter("rel_chunks", [P, c_tot], F16, isOutput=False)
    dinv_d = nc.declare_dram_parameter("dinv_tiles", [P, N_TILES], F32, isOutput=False)
    posT_d = nc.declare_dram_parameter("posT", [65, NODES_PAD], F16, isOutput=False)
    wg_d = nc.declare_dram_parameter("W_gcn", [D, D], F16, isOutput=False)
    wa_d = nc.declare_dram_parameter("W_aug", [65, D], F16, isOutput=False)
    out_d = nc.declare_dram_parameter("out", [NPC, D], F32, isOutput=True)

    eq = mybir.AluOpType.is_equal
    n_slabs = (c_tot + SLAB - 1) // SLAB

    max_ch = int(ct.max())
    with tile.TileContext(nc) as tc:
        with (
            tc.tile_pool(name="const", bufs=1) as cpool,
            tc.tile_pool(name="msg", bufs=48) as mpool,
            tc.tile_pool(name="amat", bufs=4) as apool,
            tc.tile_pool(name="small", bufs=3) as spool,
            tc.tile_pool(name="outb", bufs=3) as opool,
            tc.tile_pool(name="ps_s", bufs=2, space="PSUM") as ps_s,
            tc.tile_pool(name="ps_t", bufs=2, space="PSUM") as ps_t,
            tc.tile_pool(name="ps_o", bufs=2, space="PSUM") as ps_o,
        ):
            iota_i = cpool.tile([P, P], mybir.dt.int16)
            nc.gpsimd.iota(iota_i[:], pattern=[[1, P]], base=0,
                           channel_multiplier=0)
            iota_t = cpool.tile([P, P], F16)
            nc.vector.tensor_copy(out=iota_t[:], in_=iota_i[:])
            ident_t = cpool.tile([P, P], F16)
            make_identity(nc, ident_t[:])
            wg_t = cpool.tile([D, D], F16)
            nc.sync.dma_start(out=wg_t[:], in_=wg_d[:])
            wa_t = cpool.tile([65, D], F16)
            nc.sync.dma_start(out=wa_t[:], in_=wa_d[:])
            dinv_t = cpool.tile([P, N_TILES], F32)
            nc.sync.dma_start(out=dinv_t[:], in_=dinv_d[:])
            posT_t = cpool.tile([65, NODES_PAD], F16)
            nc.sync.dma_start(out=posT_t[:], in_=posT_d[:])
            src_all = cpool.tile([P, c_tot], I32)
            nc.sync.dma_start(out=src_all[:], in_=src_d[:])
            rel_all = cpool.tile([P, c_tot], F16)
            nc.sync.dma_start(out=rel_all[:], in_=rel_d[:])

            for t in range(N_TILES):
                psum_s = ps_s.tile([P, D], F32)
                n_ch = int(ct[t])
                j0 = int(off[t])
                a_big = apool.tile([P, max_ch, P], F16, tag="a_big")
                nc.vector.tensor_tensor(
                    out=a_big[:, :n_ch, :],
                    in0=rel_all[:, j0:j0 + n_ch].unsqueeze(2)
                        .to_broadcast([P, n_ch, P]),
                    in1=iota_t[:].unsqueeze(1).to_broadcast([P, n_ch, P]),
                    op=eq)
                for j in range(n_ch):
                    msg = mpool.tile([P, D], F16)
                    nc.gpsimd.indirect_dma_start(
                        out=msg[:], out_offset=None,
                        in_=xs_d[:],
                        in_offset=IndirectOffsetOnAxis(
                            ap=src_all[:, j0 + j:j0 + j + 1], axis=0))
                    nc.tensor.matmul(
                        out=psum_s[:], lhsT=a_big[:, j, :], rhs=msg[:],
                        start=(j == 0), stop=(j == n_ch - 1))

                s16 = spool.tile([P, D], F16, tag="s16")
                nc.scalar.mul(out=s16[:], in_=psum_s[:], mul=dinv_t[:, t:t + 1])
                psT = ps_t.tile([D, P], F16)
                nc.tensor.transpose(out=psT[:], in_=s16[:], identity=ident_t[:])
                sT = spool.tile([D, P], F16, tag="sT")
                nc.scalar.copy(out=sT[:], in_=psT[:])
                psum_o = ps_o.tile([P, D], F32)
                nc.tensor.matmul(out=psum_o[:], lhsT=sT[:], rhs=wg_t[:],
                                 start=True, stop=False)
                nc.tensor.matmul(out=psum_o[:],
                                 lhsT=posT_t[:, t * P:(t + 1) * P],
                                 rhs=wa_t[:], start=False, stop=True)
                out_sb = opool.tile([P, D], F32)
                nc.scalar.copy(out=out_sb[:], in_=psum_o[:])
                rows = min(P, NPC - t * P)
                nc.sync.dma_start(out=out_d[t * P:t * P + rows, :],
                                  in_=out_sb[:rows, :])
    nc.compile()
    return nc


def kernel(x, edge_index, pos_encoding, W_gcn, b_gcn, W_pos, b_pos,
           _trace=False, _result_box=None):
    shared, per_core, ct, off, c_tot = _preprocess(
        x, edge_index, pos_encoding, W_gcn, b_gcn, W_pos, b_pos)
    nc = _build_program(ct, off, c_tot)
    in_maps = [{**shared, **per_core[c]} for c in range(N_CORES)]
    res = run_bass_kernel_spmd(nc, in_maps, list(range(N_CORES)),
                               trace=_trace)
    if _result_box is not None:
        _result_box.append(res)
    out = np.concatenate([res.results[c]["out"] for c in range(N_CORES)], axis=0)
    return out.astype(np.float32)


if __name__ == "__main__":
    rng = np.random.default_rng(0)
    x = rng.standard_normal((N_NODES, D), dtype=np.float32)
    ei = rng.integers(0, N_NODES, size=(2, 1600000)).astype(np.int64)
    pe = rng.standard_normal((N_NODES, D), dtype=np.float32)
    Wg = rng.standard_normal((D, D), dtype=np.float32) / 8
    bg = rng.standard_normal(D, dtype=np.float32) * 0.01
    Wp = rng.standard_normal((D, D), dtype=np.float32) / 8
    bp = rng.standard_normal(D, dtype=np.float32) * 0.01
    out = kernel(x, ei, pe, Wg, bg, Wp, bp)
    print(out.shape, out.dtype)
